# revision 4
# baseline (speedup 1.0000x reference)
"""Trainium2 Bass kernel for nn_Loss_for_localization (YOLO-style loss).

Strategy: pure data parallel over the batch dim. Each of the 8 NeuronCores
processes 8 of the 64 batch images (its shard of objects/scores/locs/label/gt),
computes a scalar partial loss on-device, and the host sums the 8 partials
and divides by the batch size.

Per-core layout: for each anchor a (9 iterations), every channel plane group
objects[:, a] / gt[:, a, c] / locs[:, a, c] ([8,128,128] = 8 planes of 16384
elements) is viewed as [128, 1024]: partition p = (b_local, sixteenth-of-plane),
free dim = 1024 contiguous elements. Every DMA run is 4 KiB contiguous in
HBM. The 10 channel streams per iteration are split across BOTH HWDGE rings
(5 via the SP sequencer, 5 via the ACT sequencer) so all 16 SDMA engines
pull from HBM - a single ring only drives 8 engines (~205 GB/s measured).
All channel tiles share the same (partition, free) -> (b, cell) mapping, so
elementwise masking stays aligned.

Loss math on device (per a-iteration, fused DVE/ACT ops):
  lg0 = Ln(1-p)  [ACT, accum_out gives sum(lg0) for free]
  sum m*nl0      via one scalar_tensor_tensor: (m * -1) * lg0, accum_out
                 (the -100 clamp never binds for log1p(-p) of p in [0,1))
  nl1 = min(-Ln(p), 100)   [stt with a constant 100-tile; handles p == 0]
  sum m*nl1      via stt accum_out
  d = locs_all - gt_coords_all   [one [128,4096] sub on GpSimd]
  s = Square(d)                  [one ACT op]
  sum m*s        via one stt with m broadcast (stride-0) over the 4 channels
Scores part (tiny): max -> Exp(bias=-max, accum) -> Ln -> +max = lse;
one-hot(label) dot scores = picked; img partial = sum(lse - picked).
Final: weighted combine + cross-partition reduce via a [128,1]x[128,1]
ones-matmul into PSUM; DMA scalar out.

This file also carries a workaround for the container's walrus build, which
accepts at most ONE sync-wait and ONE sem-update per instruction: a BIR-JSON
post-pass (hooked into Bass.to_json_bytes) moves excess waits/updates onto
EventSemaphore carrier instructions on the same engine queue.
"""

import sys

sys.path.insert(0, "/opt/trn_rl_repo")

from contextlib import ExitStack

import numpy as np
import orjson

import concourse.bass as bass
import concourse.mybir as mybir
import concourse.tile as tile
from concourse.bass_utils import run_bass_kernel_spmd

f32 = mybir.dt.float32
ALU = mybir.AluOpType
ACTF = mybir.ActivationFunctionType
AX = mybir.AxisListType

N_CORES = 8
B, A, H, W, C = 64, 9, 128, 128, 1000
BL = B // N_CORES  # 8 local batches per core
S = 16             # sixteenths of a plane; partition = (b_local, s)
F = (H * W) // S   # 1024 free elements per partition

# ---------------------------------------------------------------------------
# walrus <=1 sync-wait / <=1 sem-update per instruction workaround
# ---------------------------------------------------------------------------
_split_counter = [0]


def _carrier(engine, debug, sync_info):
    _split_counter[0] += 1
    return {
        "opcode": "EventSemaphore",
        "engine": engine,
        "ins": [],
        "outs": [],
        "name": f"splitsync-{_split_counter[0]}",
        "debug": debug,
        "sync_info": sync_info,
    }


def _split_excess_sync_json(bir: bytes) -> bytes:
    m = orjson.loads(bir)
    changed = False
    for fn in m.get("functions", []):
        for bb in fn.get("blocks", []):
            instrs = bb.get("instructions")
            if not instrs:
                continue
            out = []
            for ins in instrs:
                si = ins.get("sync_info")
                followers = []
                if si:
                    waits = si.get("on_wait") or []
                    if len(waits) > 1:
                        changed = True
                        for w in waits[:-1]:
                            out.append(
                                _carrier(ins["engine"], ins.get("debug"),
                                         {"on_wait": [w], "on_update": []})
                            )
                        si["on_wait"] = waits[-1:]
                    ups = si.get("on_update") or []
                    if len(ups) > 1:
                        assert ins.get("opcode") != "DMACopy", (
                            "cannot split sem updates off an async DMACopy"
                        )
                        changed = True
                        for u in ups[1:]:
                            followers.append(
                                _carrier(ins["engine"], ins.get("debug"),
                                         {"on_wait": [], "on_update": [u]})
                            )
                        si["on_update"] = ups[:1]
                out.append(ins)
                out.extend(followers)
            bb["instructions"] = out
    if not changed:
        return bir
    return orjson.dumps(m)


if not getattr(bass.Bass, "_sync_split_patched", False):
    _orig_to_json_bytes = bass.Bass.to_json_bytes

    def _patched_to_json_bytes(self):
        return _split_excess_sync_json(_orig_to_json_bytes(self))

    bass.Bass.to_json_bytes = _patched_to_json_bytes
    bass.Bass._sync_split_patched = True


# ---------------------------------------------------------------------------
# program builder
# ---------------------------------------------------------------------------
def _plane_ap(t):
    """[8,128,128] channel-plane AP -> [8,16,1024] walked as [128, 1024].

    Partition p = b_local*16 + sixteenth; every partition's elements are one
    contiguous 4 KiB run in HBM. The DMA walks the 3D DRAM AP in the same
    element order as the 2D SBUF tile (partition-major), so shapes need not
    match dim-for-dim (validated by roundtrip on hardware).
    """
    return t.rearrange("b (s f1) w -> b s (f1 w)", s=S)


def _by_channel(ap, c=4):
    return ap.rearrange("p (c f) -> p c f", c=c)


def _build_program():
    nc = bass.Bass()
    objects_l = nc.declare_dram_parameter("objects", [BL, A, H, W], f32, isOutput=False)
    scores_l = nc.declare_dram_parameter("scores", [BL, C], f32, isOutput=False)
    locs_l = nc.declare_dram_parameter("locs", [BL, A, 4, H, W], f32, isOutput=False)
    label_l = nc.declare_dram_parameter("label", [BL, 1], f32, isOutput=False)
    gt_l = nc.declare_dram_parameter("gt", [BL, A, 5, H, W], f32, isOutput=False)
    out_d = nc.declare_dram_parameter("out", [1, 1], f32, isOutput=True)

    with tile.TileContext(nc) as tc, ExitStack() as octx:
        # long-lived state
        fin = octx.enter_context(tc.tile_pool(name="fin", bufs=1))
        hundred = fin.tile([128, F], f32)
        nc.vector.memset(hundred[:], 100.0)
        acc_lg0 = fin.tile([128, A], f32)
        acc_mnl0 = fin.tile([128, A], f32)
        acc_mnl1 = fin.tile([128, A], f32)
        acc_msq = fin.tile([128, A], f32)
        nc.vector.memset(acc_lg0[:], 0.0)
        nc.vector.memset(acc_mnl0[:], 0.0)
        nc.vector.memset(acc_mnl1[:], 0.0)
        nc.vector.memset(acc_msq[:], 0.0)

        with ExitStack() as ctx:
            inp = ctx.enter_context(tc.tile_pool(name="inp", bufs=2))
            mid = ctx.enter_context(tc.tile_pool(name="mid", bufs=2))
            dsp = ctx.enter_context(tc.tile_pool(name="dsp", bufs=2))
            jnk = ctx.enter_context(tc.tile_pool(name="jnk", bufs=2))
            pjk = ctx.enter_context(tc.tile_pool(name="pjk", bufs=1, space="PSUM"))

            for a in range(A):
                # --- loads: 5 streams per HWDGE ring ---
                p_t = inp.tile([128, F], f32, tag="p")
                nc.sync.dma_start(p_t[:], _plane_ap(objects_l[:, a]))
                l_all = inp.tile([128, 4 * F], f32, tag="l")
                for c in range(4):
                    nc.sync.dma_start(l_all[:, c * F:(c + 1) * F],
                                      _plane_ap(locs_l[:, a, c]))
                m_t = inp.tile([128, F], f32, tag="m")
                nc.scalar.dma_start(m_t[:], _plane_ap(gt_l[:, a, 0]))
                g_all = inp.tile([128, 4 * F], f32, tag="g")
                for c in range(4):
                    nc.scalar.dma_start(g_all[:, c * F:(c + 1) * F],
                                        _plane_ap(gt_l[:, a, 1 + c]))

                # --- objectness BCE terms ---
                lg0 = mid.tile([128, F], f32, tag="lg0")
                nc.scalar.activation(lg0[:], p_t[:], ACTF.Ln, bias=1.0, scale=-1.0,
                                     accum_out=acc_lg0[:, a:a + 1])
                lg1 = mid.tile([128, F], f32, tag="lg1")
                nc.scalar.activation(lg1[:], p_t[:], ACTF.Ln)

                jk = jnk.tile([128, F], f32, tag="junk")
                nc.vector.scalar_tensor_tensor(
                    out=jk[:], in0=m_t[:], scalar=-1.0, in1=lg0[:],
                    op0=ALU.mult, op1=ALU.mult, accum_out=acc_mnl0[:, a:a + 1])

                nl1 = mid.tile([128, F], f32, tag="nl1")
                nc.vector.scalar_tensor_tensor(
                    out=nl1[:], in0=lg1[:], scalar=-1.0, in1=hundred[:],
                    op0=ALU.mult, op1=ALU.min)
                jk = jnk.tile([128, F], f32, tag="junk")
                nc.vector.scalar_tensor_tensor(
                    out=jk[:], in0=m_t[:], scalar=1.0, in1=nl1[:],
                    op0=ALU.mult, op1=ALU.mult, accum_out=acc_mnl1[:, a:a + 1])

                # --- coordinate L2 term ---
                d_all = dsp.tile([128, 4 * F], f32, tag="d")
                nc.gpsimd.tensor_tensor(out=d_all[:], in0=l_all[:], in1=g_all[:],
                                        op=ALU.subtract)
                s_all = dsp.tile([128, 4 * F], f32, tag="s")
                nc.scalar.activation(s_all[:], d_all[:], ACTF.Square)
                m_bc = m_t[:].rearrange("p (c f) -> p c f", c=1).broadcast_to(
                    (128, 4, F))
                pj = pjk.tile([128, 4 * F], f32, tag="pjunk")
                nc.vector.scalar_tensor_tensor(
                    out=_by_channel(pj[:]), in0=m_bc, scalar=1.0,
                    in1=_by_channel(s_all[:]),
                    op0=ALU.mult, op1=ALU.mult, accum_out=acc_msq[:, a:a + 1])

        # ---- per-partition reductions & weighted combine ----
        s_lg0 = fin.tile([128, 1], f32)
        s_mnl0 = fin.tile([128, 1], f32)
        s_mnl1 = fin.tile([128, 1], f32)
        s_msq = fin.tile([128, 1], f32)
        nc.vector.tensor_reduce(out=s_lg0[:], in_=acc_lg0[:], axis=AX.X, op=ALU.add)
        nc.vector.tensor_reduce(out=s_mnl0[:], in_=acc_mnl0[:], axis=AX.X, op=ALU.add)
        nc.vector.tensor_reduce(out=s_mnl1[:], in_=acc_mnl1[:], axis=AX.X, op=ALU.add)
        nc.vector.tensor_reduce(out=s_msq[:], in_=acc_msq[:], axis=AX.X, op=ALU.add)

        # loss_vec = 0.5*(Snl0 - Smnl0) + Smnl1 + 5*Smsq ; Snl0 = -s_lg0
        t1 = fin.tile([128, 1], f32)
        nc.vector.scalar_tensor_tensor(out=t1[:], in0=s_lg0[:], scalar=1.0,
                                       in1=s_mnl0[:], op0=ALU.mult, op1=ALU.add)
        t2 = fin.tile([128, 1], f32)
        nc.vector.scalar_tensor_tensor(out=t2[:], in0=t1[:], scalar=-0.5,
                                       in1=s_mnl1[:], op0=ALU.mult, op1=ALU.add)
        t3 = fin.tile([128, 1], f32)
        nc.vector.scalar_tensor_tensor(out=t3[:], in0=s_msq[:], scalar=5.0,
                                       in1=t2[:], op0=ALU.mult, op1=ALU.add)

        # ---- image-classification cross entropy on this core's 8 rows ----
        with ExitStack() as sctx:
            sco = sctx.enter_context(tc.tile_pool(name="sco", bufs=1))
            psum = sctx.enter_context(tc.tile_pool(name="psum", bufs=1, space="PSUM"))

            sct = sco.tile([BL, C], f32)
            nc.sync.dma_start(sct[:], scores_l[:])
            labt = sco.tile([BL, 1], f32)
            nc.sync.dma_start(labt[:], label_l[:])

            mx = sco.tile([BL, 1], f32)
            nc.vector.tensor_reduce(out=mx[:], in_=sct[:], axis=AX.X, op=ALU.max)
            nmx = sco.tile([BL, 1], f32)
            nc.vector.tensor_scalar(out=nmx[:], in0=mx[:], scalar1=-1.0,
                                    scalar2=None, op0=ALU.mult)
            et = sco.tile([BL, C], f32)
            se = sco.tile([BL, 1], f32)
            nc.scalar.activation(et[:], sct[:], ACTF.Exp, bias=nmx[:], scale=1.0,
                                 accum_out=se[:])
            lse0 = sco.tile([BL, 1], f32)
            nc.scalar.activation(lse0[:], se[:], ACTF.Ln)
            lse = sco.tile([BL, 1], f32)
            nc.vector.tensor_tensor(out=lse[:], in0=lse0[:], in1=mx[:], op=ALU.add)

            io = sco.tile([BL, C], mybir.dt.int32)
            nc.gpsimd.iota(io[:], pattern=[[1, C]], base=0, channel_multiplier=0)
            iof = sco.tile([BL, C], f32)
            nc.vector.tensor_copy(iof[:], io[:])
            oh = sco.tile([BL, C], f32)
            nc.vector.tensor_scalar(out=oh[:], in0=iof[:], scalar1=labt[:],
                                    scalar2=None, op0=ALU.is_equal)
            jk2 = sco.tile([BL, C], f32)
            pk = sco.tile([BL, 1], f32)
            nc.vector.scalar_tensor_tensor(out=jk2[:], in0=oh[:], scalar=1.0,
                                           in1=sct[:], op0=ALU.mult, op1=ALU.mult,
                                           accum_out=pk[:])
            img = sco.tile([BL, 1], f32)
            nc.vector.scalar_tensor_tensor(out=img[:], in0=pk[:], scalar=-1.0,
                                           in1=lse[:], op0=ALU.mult, op1=ALU.add)

            # ---- cross-partition reduce via ones-matmul into PSUM ----
            ones = sco.tile([128, 1], f32)
            nc.vector.memset(ones[:], 1.0)
            acc = psum.tile([1, 1], f32)
            nc.tensor.matmul(acc[:], ones[:], t3[:], start=True, stop=False)
            nc.tensor.matmul(acc[:], ones[:BL, :], img[:], start=False, stop=True)

            res = sco.tile([1, 1], f32)
            nc.scalar.copy(out=res[:], in_=acc[:])
            nc.sync.dma_start(out_d[:], res[:])

    return nc


_program_cache = {}


def _get_program():
    if "nc" not in _program_cache:
        _program_cache["nc"] = _build_program()
    return _program_cache["nc"]


def kernel(objects, scores, locs, label, gt, _trace=False, _trace_kwargs=None):
    objects = np.ascontiguousarray(np.asarray(objects, dtype=np.float32))
    scores = np.ascontiguousarray(np.asarray(scores, dtype=np.float32))
    locs = np.ascontiguousarray(np.asarray(locs, dtype=np.float32))
    gt = np.ascontiguousarray(np.asarray(gt, dtype=np.float32))
    labf = np.asarray(label).astype(np.float32).reshape(B, 1)

    nc = _get_program()
    in_maps = []
    for i in range(N_CORES):
        sl = slice(i * BL, (i + 1) * BL)
        in_maps.append({
            "objects": objects[sl],
            "scores": scores[sl],
            "locs": locs[sl],
            "label": np.ascontiguousarray(labf[sl]),
            "gt": gt[sl],
        })

    kw = {}
    if _trace:
        kw["trace"] = True
        kw.update(_trace_kwargs or {})
    res = run_bass_kernel_spmd(nc, in_maps, list(range(N_CORES)), **kw)
    partials = [float(res.results[i]["out"][0, 0]) for i in range(N_CORES)]
    total = np.float32(np.sum(np.asarray(partials, dtype=np.float64)) / B)
    out = np.array(total, dtype=np.float32)
    if _trace:
        return out, res
    return out


# revision 5
# speedup vs baseline: 1.0502x; 1.0502x over previous
"""Trainium2 Bass kernel for nn_Loss_for_localization (YOLO-style loss).

Strategy: pure data parallel over the batch dim. Each of the 8 NeuronCores
processes 8 of the 64 batch images (its shard of objects/scores/locs/label/gt),
computes a scalar partial loss on-device, and the host sums the 8 partials
and divides by the batch size.

Per-core layout: for each anchor a (9 iterations), every channel plane group
objects[:, a] / gt[:, a, c] / locs[:, a, c] ([8,128,128] = 8 planes of 16384
elements) is viewed as [128, 1024]: partition p = (b_local, sixteenth-of-plane),
free dim = 1024 contiguous elements. Every DMA run is 4 KiB contiguous in
HBM. All 10 channel streams per iteration are issued on the SP HWDGE ring;
this virtual core exposes 8 SDMA engines (E64-71, ~232 GB/s line rate),
and a single busy ring already saturates them (~205 GB/s measured; a
dual-ring split only added contention on the same engines).
All channel tiles share the same (partition, free) -> (b, cell) mapping, so
elementwise masking stays aligned.

Loss math on device (per a-iteration, fused DVE/ACT ops):
  lg0 = Ln(1-p)  [ACT, accum_out gives sum(lg0) for free]
  sum m*nl0      via one scalar_tensor_tensor: (m * -1) * lg0, accum_out
                 (the -100 clamp never binds for log1p(-p) of p in [0,1))
  nl1 = min(-Ln(p), 100)   [stt with a constant 100-tile; handles p == 0]
  sum m*nl1      via stt accum_out
  d = locs_all - gt_coords_all   [one [128,4096] sub on GpSimd]
  s = Square(d)                  [one ACT op]
  sum m*s        via one stt with m broadcast (stride-0) over the 4 channels
Scores part (tiny): max -> Exp(bias=-max, accum) -> Ln -> +max = lse;
one-hot(label) dot scores = picked; img partial = sum(lse - picked).
Final: weighted combine + cross-partition reduce via a [128,1]x[128,1]
ones-matmul into PSUM; DMA scalar out.

This file also carries a workaround for the container's walrus build, which
accepts at most ONE sync-wait and ONE sem-update per instruction: a BIR-JSON
post-pass (hooked into Bass.to_json_bytes) moves excess waits/updates onto
EventSemaphore carrier instructions on the same engine queue.
"""

import sys

sys.path.insert(0, "/opt/trn_rl_repo")

from contextlib import ExitStack

import numpy as np
import orjson

import concourse.bass as bass
import concourse.mybir as mybir
import concourse.tile as tile
from concourse.bass_utils import run_bass_kernel_spmd

f32 = mybir.dt.float32
ALU = mybir.AluOpType
ACTF = mybir.ActivationFunctionType
AX = mybir.AxisListType

N_CORES = 8
B, A, H, W, C = 64, 9, 128, 128, 1000
BL = B // N_CORES  # 8 local batches per core
S = 16             # sixteenths of a plane; partition = (b_local, s)
F = (H * W) // S   # 1024 free elements per partition

# ---------------------------------------------------------------------------
# walrus <=1 sync-wait / <=1 sem-update per instruction workaround
# ---------------------------------------------------------------------------
_split_counter = [0]


def _carrier(engine, debug, sync_info):
    _split_counter[0] += 1
    return {
        "opcode": "EventSemaphore",
        "engine": engine,
        "ins": [],
        "outs": [],
        "name": f"splitsync-{_split_counter[0]}",
        "debug": debug,
        "sync_info": sync_info,
    }


def _split_excess_sync_json(bir: bytes) -> bytes:
    m = orjson.loads(bir)
    changed = False
    for fn in m.get("functions", []):
        for bb in fn.get("blocks", []):
            instrs = bb.get("instructions")
            if not instrs:
                continue
            out = []
            for ins in instrs:
                si = ins.get("sync_info")
                followers = []
                if si:
                    waits = si.get("on_wait") or []
                    if len(waits) > 1:
                        changed = True
                        for w in waits[:-1]:
                            out.append(
                                _carrier(ins["engine"], ins.get("debug"),
                                         {"on_wait": [w], "on_update": []})
                            )
                        si["on_wait"] = waits[-1:]
                    ups = si.get("on_update") or []
                    if len(ups) > 1:
                        assert ins.get("opcode") != "DMACopy", (
                            "cannot split sem updates off an async DMACopy"
                        )
                        changed = True
                        for u in ups[1:]:
                            followers.append(
                                _carrier(ins["engine"], ins.get("debug"),
                                         {"on_wait": [], "on_update": [u]})
                            )
                        si["on_update"] = ups[:1]
                out.append(ins)
                out.extend(followers)
            bb["instructions"] = out
    if not changed:
        return bir
    return orjson.dumps(m)


if not getattr(bass.Bass, "_sync_split_patched", False):
    _orig_to_json_bytes = bass.Bass.to_json_bytes

    def _patched_to_json_bytes(self):
        return _split_excess_sync_json(_orig_to_json_bytes(self))

    bass.Bass.to_json_bytes = _patched_to_json_bytes
    bass.Bass._sync_split_patched = True


# ---------------------------------------------------------------------------
# program builder
# ---------------------------------------------------------------------------
def _plane_ap(t):
    """[8,128,128] channel-plane AP -> [8,16,1024] walked as [128, 1024].

    Partition p = b_local*16 + sixteenth; every partition's elements are one
    contiguous 4 KiB run in HBM. The DMA walks the 3D DRAM AP in the same
    element order as the 2D SBUF tile (partition-major), so shapes need not
    match dim-for-dim (validated by roundtrip on hardware).
    """
    return t.rearrange("b (s f1) w -> b s (f1 w)", s=S)


def _by_channel(ap, c=4):
    return ap.rearrange("p (c f) -> p c f", c=c)


def _build_program():
    nc = bass.Bass()
    objects_l = nc.declare_dram_parameter("objects", [BL, A, H, W], f32, isOutput=False)
    scores_l = nc.declare_dram_parameter("scores", [BL, C], f32, isOutput=False)
    locs_l = nc.declare_dram_parameter("locs", [BL, A, 4, H, W], f32, isOutput=False)
    label_l = nc.declare_dram_parameter("label", [BL, 1], f32, isOutput=False)
    gt_l = nc.declare_dram_parameter("gt", [BL, A, 5, H, W], f32, isOutput=False)
    out_d = nc.declare_dram_parameter("out", [1, 1], f32, isOutput=True)

    with tile.TileContext(nc) as tc, ExitStack() as octx:
        # long-lived state
        fin = octx.enter_context(tc.tile_pool(name="fin", bufs=1))
        hundred = fin.tile([128, F], f32)
        nc.vector.memset(hundred[:], 100.0)
        acc_lg0 = fin.tile([128, A], f32)
        acc_mnl0 = fin.tile([128, A], f32)
        acc_mnl1 = fin.tile([128, A], f32)
        acc_msq = fin.tile([128, A], f32)
        nc.vector.memset(acc_lg0[:], 0.0)
        nc.vector.memset(acc_mnl0[:], 0.0)
        nc.vector.memset(acc_mnl1[:], 0.0)
        nc.vector.memset(acc_msq[:], 0.0)

        with ExitStack() as ctx:
            inp = ctx.enter_context(tc.tile_pool(name="inp", bufs=2))
            mid = ctx.enter_context(tc.tile_pool(name="mid", bufs=2))
            dsp = ctx.enter_context(tc.tile_pool(name="dsp", bufs=2))
            jnk = ctx.enter_context(tc.tile_pool(name="jnk", bufs=2))
            pjk = ctx.enter_context(tc.tile_pool(name="pjk", bufs=1, space="PSUM"))

            for a in range(A):
                # --- loads: 5 streams per HWDGE ring ---
                p_t = inp.tile([128, F], f32, tag="p")
                nc.sync.dma_start(p_t[:], _plane_ap(objects_l[:, a]))
                l_all = inp.tile([128, 4 * F], f32, tag="l")
                for c in range(4):
                    nc.sync.dma_start(l_all[:, c * F:(c + 1) * F],
                                      _plane_ap(locs_l[:, a, c]))
                m_t = inp.tile([128, F], f32, tag="m")
                nc.sync.dma_start(m_t[:], _plane_ap(gt_l[:, a, 0]))
                g_all = inp.tile([128, 4 * F], f32, tag="g")
                for c in range(4):
                    nc.sync.dma_start(g_all[:, c * F:(c + 1) * F],
                                      _plane_ap(gt_l[:, a, 1 + c]))

                # --- objectness BCE terms ---
                lg0 = mid.tile([128, F], f32, tag="lg0")
                nc.scalar.activation(lg0[:], p_t[:], ACTF.Ln, bias=1.0, scale=-1.0,
                                     accum_out=acc_lg0[:, a:a + 1])
                lg1 = mid.tile([128, F], f32, tag="lg1")
                nc.scalar.activation(lg1[:], p_t[:], ACTF.Ln)

                jk = jnk.tile([128, F], f32, tag="junk")
                nc.vector.scalar_tensor_tensor(
                    out=jk[:], in0=m_t[:], scalar=-1.0, in1=lg0[:],
                    op0=ALU.mult, op1=ALU.mult, accum_out=acc_mnl0[:, a:a + 1])

                nl1 = mid.tile([128, F], f32, tag="nl1")
                nc.vector.scalar_tensor_tensor(
                    out=nl1[:], in0=lg1[:], scalar=-1.0, in1=hundred[:],
                    op0=ALU.mult, op1=ALU.min)
                jk = jnk.tile([128, F], f32, tag="junk")
                nc.vector.scalar_tensor_tensor(
                    out=jk[:], in0=m_t[:], scalar=1.0, in1=nl1[:],
                    op0=ALU.mult, op1=ALU.mult, accum_out=acc_mnl1[:, a:a + 1])

                # --- coordinate L2 term ---
                d_all = dsp.tile([128, 4 * F], f32, tag="d")
                nc.gpsimd.tensor_tensor(out=d_all[:], in0=l_all[:], in1=g_all[:],
                                        op=ALU.subtract)
                s_all = dsp.tile([128, 4 * F], f32, tag="s")
                nc.scalar.activation(s_all[:], d_all[:], ACTF.Square)
                m_bc = m_t[:].rearrange("p (c f) -> p c f", c=1).broadcast_to(
                    (128, 4, F))
                pj = pjk.tile([128, 4 * F], f32, tag="pjunk")
                nc.vector.scalar_tensor_tensor(
                    out=_by_channel(pj[:]), in0=m_bc, scalar=1.0,
                    in1=_by_channel(s_all[:]),
                    op0=ALU.mult, op1=ALU.mult, accum_out=acc_msq[:, a:a + 1])

        # ---- per-partition reductions & weighted combine ----
        s_lg0 = fin.tile([128, 1], f32)
        s_mnl0 = fin.tile([128, 1], f32)
        s_mnl1 = fin.tile([128, 1], f32)
        s_msq = fin.tile([128, 1], f32)
        nc.vector.tensor_reduce(out=s_lg0[:], in_=acc_lg0[:], axis=AX.X, op=ALU.add)
        nc.vector.tensor_reduce(out=s_mnl0[:], in_=acc_mnl0[:], axis=AX.X, op=ALU.add)
        nc.vector.tensor_reduce(out=s_mnl1[:], in_=acc_mnl1[:], axis=AX.X, op=ALU.add)
        nc.vector.tensor_reduce(out=s_msq[:], in_=acc_msq[:], axis=AX.X, op=ALU.add)

        # loss_vec = 0.5*(Snl0 - Smnl0) + Smnl1 + 5*Smsq ; Snl0 = -s_lg0
        t1 = fin.tile([128, 1], f32)
        nc.vector.scalar_tensor_tensor(out=t1[:], in0=s_lg0[:], scalar=1.0,
                                       in1=s_mnl0[:], op0=ALU.mult, op1=ALU.add)
        t2 = fin.tile([128, 1], f32)
        nc.vector.scalar_tensor_tensor(out=t2[:], in0=t1[:], scalar=-0.5,
                                       in1=s_mnl1[:], op0=ALU.mult, op1=ALU.add)
        t3 = fin.tile([128, 1], f32)
        nc.vector.scalar_tensor_tensor(out=t3[:], in0=s_msq[:], scalar=5.0,
                                       in1=t2[:], op0=ALU.mult, op1=ALU.add)

        # ---- image-classification cross entropy on this core's 8 rows ----
        with ExitStack() as sctx:
            sco = sctx.enter_context(tc.tile_pool(name="sco", bufs=1))
            psum = sctx.enter_context(tc.tile_pool(name="psum", bufs=1, space="PSUM"))

            sct = sco.tile([BL, C], f32)
            nc.sync.dma_start(sct[:], scores_l[:])
            labt = sco.tile([BL, 1], f32)
            nc.sync.dma_start(labt[:], label_l[:])

            mx = sco.tile([BL, 1], f32)
            nc.vector.tensor_reduce(out=mx[:], in_=sct[:], axis=AX.X, op=ALU.max)
            nmx = sco.tile([BL, 1], f32)
            nc.vector.tensor_scalar(out=nmx[:], in0=mx[:], scalar1=-1.0,
                                    scalar2=None, op0=ALU.mult)
            et = sco.tile([BL, C], f32)
            se = sco.tile([BL, 1], f32)
            nc.scalar.activation(et[:], sct[:], ACTF.Exp, bias=nmx[:], scale=1.0,
                                 accum_out=se[:])
            lse0 = sco.tile([BL, 1], f32)
            nc.scalar.activation(lse0[:], se[:], ACTF.Ln)
            lse = sco.tile([BL, 1], f32)
            nc.vector.tensor_tensor(out=lse[:], in0=lse0[:], in1=mx[:], op=ALU.add)

            io = sco.tile([BL, C], mybir.dt.int32)
            nc.gpsimd.iota(io[:], pattern=[[1, C]], base=0, channel_multiplier=0)
            iof = sco.tile([BL, C], f32)
            nc.vector.tensor_copy(iof[:], io[:])
            oh = sco.tile([BL, C], f32)
            nc.vector.tensor_scalar(out=oh[:], in0=iof[:], scalar1=labt[:],
                                    scalar2=None, op0=ALU.is_equal)
            jk2 = sco.tile([BL, C], f32)
            pk = sco.tile([BL, 1], f32)
            nc.vector.scalar_tensor_tensor(out=jk2[:], in0=oh[:], scalar=1.0,
                                           in1=sct[:], op0=ALU.mult, op1=ALU.mult,
                                           accum_out=pk[:])
            img = sco.tile([BL, 1], f32)
            nc.vector.scalar_tensor_tensor(out=img[:], in0=pk[:], scalar=-1.0,
                                           in1=lse[:], op0=ALU.mult, op1=ALU.add)

            # ---- cross-partition reduce via ones-matmul into PSUM ----
            ones = sco.tile([128, 1], f32)
            nc.vector.memset(ones[:], 1.0)
            acc = psum.tile([1, 1], f32)
            nc.tensor.matmul(acc[:], ones[:], t3[:], start=True, stop=False)
            nc.tensor.matmul(acc[:], ones[:BL, :], img[:], start=False, stop=True)

            res = sco.tile([1, 1], f32)
            nc.scalar.copy(out=res[:], in_=acc[:])
            nc.sync.dma_start(out_d[:], res[:])

    return nc


_program_cache = {}


def _get_program():
    if "nc" not in _program_cache:
        _program_cache["nc"] = _build_program()
    return _program_cache["nc"]


def kernel(objects, scores, locs, label, gt, _trace=False, _trace_kwargs=None):
    objects = np.ascontiguousarray(np.asarray(objects, dtype=np.float32))
    scores = np.ascontiguousarray(np.asarray(scores, dtype=np.float32))
    locs = np.ascontiguousarray(np.asarray(locs, dtype=np.float32))
    gt = np.ascontiguousarray(np.asarray(gt, dtype=np.float32))
    labf = np.asarray(label).astype(np.float32).reshape(B, 1)

    nc = _get_program()
    in_maps = []
    for i in range(N_CORES):
        sl = slice(i * BL, (i + 1) * BL)
        in_maps.append({
            "objects": objects[sl],
            "scores": scores[sl],
            "locs": locs[sl],
            "label": np.ascontiguousarray(labf[sl]),
            "gt": gt[sl],
        })

    kw = {}
    if _trace:
        kw["trace"] = True
        kw.update(_trace_kwargs or {})
    res = run_bass_kernel_spmd(nc, in_maps, list(range(N_CORES)), **kw)
    partials = [float(res.results[i]["out"][0, 0]) for i in range(N_CORES)]
    total = np.float32(np.sum(np.asarray(partials, dtype=np.float64)) / B)
    out = np.array(total, dtype=np.float32)
    if _trace:
        return out, res
    return out
